# revision 9
# baseline (speedup 1.0000x reference)
"""Trainium2 Bass kernel for nn_DGC_Attention (global-context attention block).

Math (per batch b):
    cm[s]   = sum_c x[b,c,s] * wm[c]            (+ bm, which cancels in softmax)
    mask[s] = softmax(cm)[s] + 1/S              (uniform part: softmax of zeros)
    ctx[c]  = sum_s x[b,c,s] * mask[s]
    t       = relu(LN(ctx @ w1.T + b1) * ln_g + ln_b)
    out     = t @ w2.T + b2                     -> [B, C, 1, 1]

Sharding: pure data parallel, batch dim (16) over 8 cores, 2 batches/core.

Pipeline (v3): x streams in (b, s-slice) phases: b0 [2048u, 2048z],
b1 [2048u, 1024u, 1024z].  Per phase:
    - 8 chunk DMAs [128, ss] fp32 (HWDGE single queue; 1MB / 0.5MB each).
    - PE: cm partial via f32r matmuls -> [1, ss] PSUM.
    - 'u' phases: ACT per chunk Copy -> junk bf16 with accum_out = rowsum
      column (uniform part), as chunks arrive; fully hidden under DMA.
    - ACT phase end: Exp over PSUM row -> e bf16 [1, ss], accum_out = Z.
    - PE: broadcast e to eB [128, ss] f32 in PSUM via ones-matmul (fast,
      replaces the slow gpsimd partition_broadcast).
    - DVE per chunk: stt (eB + beta) * x, accum_out -> ctx_e column.
      'z' phases (the last slice of each batch) use beta = Z_batch/S as a
      per-partition scalar AP, folding the uniform part into the same pass
      (no rowsums); 'u' phases use beta = 0.
    - The final z-phase splits its 8 chunk-stts 6 on DVE + 2 on GPSIMD to
      shorten the post-DMA backlog.
Tail: ctx = ctx_e/Z (+ rowsums/S for u phases), tiny MLP in [2, 64] layout
(LN stats via ACT accum), PE transpose, w2 matmul with bias as 65th row.
"""
import numpy as np

B_PER_CORE = 2
N_CORES = 8
C = 1024
S = 4096
R = 64
NCHUNK = C // 128           # 8 c-chunks
LN_EPS = 1e-5

# (batch, s0, ss, kind) kind: 'u' = rowsum uniform part, 'z' = Z/S-folded
PHASES = [
    (0, 0, 2048, 'u'), (0, 2048, 2048, 'z'),
    (1, 0, 2048, 'u'), (1, 2048, 1024, 'u'), (1, 3072, 1024, 'z'),
]
NPH = len(PHASES)
N_GPS = 0                   # gpsimd cannot run stt on trn2 (Pool ISA rejects it)

_CACHE = {}


def _build():
    import concourse.bass as bass
    import concourse.tile as tile
    from concourse import bacc, mybir, bass_isa

    f32 = mybir.dt.float32
    f32r = mybir.dt.float32r
    bf16 = mybir.dt.bfloat16
    AF = mybir.ActivationFunctionType
    ALU = mybir.AluOpType

    nc = bacc.Bacc("TRN2", target_bir_lowering=False, debug=False, num_devices=N_CORES)

    x_d = nc.dram_tensor("x", [B_PER_CORE, C, S], f32, kind="ExternalInput").ap()
    wmT_d = nc.dram_tensor("wmT", [128, NCHUNK], f32, kind="ExternalInput").ap()
    w1t_d = nc.dram_tensor("w1t", [128, NCHUNK * R], bf16, kind="ExternalInput").ap()
    # w2tb[r, c] for r<64 = w2[c, r]; row 64 = b2  (bias folded into matmul)
    w2tb_d = nc.dram_tensor("w2tb", [R + 1, C], bf16, kind="ExternalInput").ap()
    # [2, 3R] broadcast rows: b1 | ln_g | ln_b
    vecs_d = nc.dram_tensor("vecs", [B_PER_CORE, 3 * R], f32, kind="ExternalInput").ap()
    eye_d = nc.dram_tensor("eye2", [B_PER_CORE, B_PER_CORE], bf16, kind="ExternalInput").ap()
    out_d = nc.dram_tensor("out", [B_PER_CORE, C], f32, kind="ExternalOutput").ap()

    with tile.TileContext(nc) as tc:
        with (
            tc.tile_pool(name="xp", bufs=20) as xp,
            tc.tile_pool(name="cp", bufs=1) as cp,
            tc.tile_pool(name="wp", bufs=1) as wp,
            tc.tile_pool(name="ps", bufs=1, space="PSUM") as ps,
        ):
            # weights: wmT first (tiny, unblocks PE), rest after
            wmT = cp.tile([128, NCHUNK], f32r, tag="wmT")
            nc.gpsimd.dma_start(wmT[:], wmT_d.bitcast(f32r))
            w1t = cp.tile([128, NCHUNK * R], bf16, tag="w1t")
            nc.gpsimd.dma_start(w1t[:], w1t_d)
            w2tb = cp.tile([R + 1, C], bf16, tag="w2tb")
            nc.gpsimd.dma_start(w2tb[:], w2tb_d)
            vecs = cp.tile([B_PER_CORE, 3 * R], f32, tag="vecs")
            nc.gpsimd.dma_start(vecs[:], vecs_d)
            eye2 = cp.tile([B_PER_CORE, B_PER_CORE], bf16, tag="eye2")
            nc.gpsimd.dma_start(eye2[:], eye_d)
            b1r = vecs[:, 0:R]
            lngr = vecs[:, R:2 * R]
            lnbr = vecs[:, 2 * R:3 * R]

            # accumulator columns, col = 8*ph + k
            ctx_e = wp.tile([128, NPH * NCHUNK], f32, tag="ctx_e")
            ctx_u = wp.tile([128, NPH * NCHUNK], f32, tag="ctx_u")
            zs = wp.tile([1, NPH], f32, tag="zs")
            zb = wp.tile([1, B_PER_CORE], f32, tag="zb")
            bcol = wp.tile([128, B_PER_CORE], f32, tag="bcol")

            ones16 = wp.tile([1, 128], bf16, tag="ones16")
            nc.vector.memset(ones16[:], 1.0)

            # PE warm-up (fp32 so any-N is legal) + ACT table warm-ups
            dum = ps.tile([1, 1], f32, tag="cm")
            nc.tensor.matmul(dum[:], wmT[:, :1].bitcast(f32), wmT[:, :1].bitcast(f32),
                             start=True, stop=True)
            ewarm = wp.tile([1, 1], f32, tag="ewarm")
            nc.scalar.activation(ewarm[:], zs[:, :1], AF.Exp)
            swarm = wp.tile([1, 1], f32, tag="swarm")
            nc.scalar.sqrt(swarm[:], ewarm[:])

            scr = wp.tile([128, 2048], bf16, tag="scr")    # DVE stt junk out
            junk = wp.tile([128, 2048], bf16, tag="junk")  # ACT rowsum junk out

            for ph, (b, s0, ss, kind) in enumerate(PHASES):
                last = (ph == NPH - 1)
                xt = []
                for k in range(NCHUNK):
                    t = xp.tile([128, 2048], f32r, tag="x")
                    nc.sync.dma_start(
                        t[:, :ss],
                        x_d[b, 128 * k: 128 * (k + 1), s0: s0 + ss].bitcast(f32r),
                    )
                    xt.append(t)

                cm = ps.tile([1, 2048], f32, tag="cm")
                for k in range(NCHUNK):
                    for j in range(ss // 512):
                        nc.tensor.matmul(
                            cm[:, 512 * j: 512 * (j + 1)],
                            wmT[:, k: k + 1],
                            xt[k][:, 512 * j: 512 * (j + 1)],
                            start=(k == 0),
                            stop=(k == NCHUNK - 1),
                        )
                    if kind == 'u':
                        # uniform-part rowsum, as chunks arrive
                        nc.scalar.activation(
                            junk[:, :ss], xt[k][:, :ss].bitcast(f32), AF.Copy,
                            accum_out=ctx_u[:, 8 * ph + k: 8 * ph + k + 1],
                        )

                e16 = wp.tile([1, 2048], bf16, tag=f"e16_{ph % 2}")
                nc.scalar.activation(
                    e16[:, :ss], cm[:, :ss], AF.Exp,
                    accum_out=zs[:, ph: ph + 1],
                )
                if kind == 'z':
                    # beta = Z_batch / S, as a [128,1] scalar column
                    zsum = wp.tile([1, 1], f32, tag=f"zsum{b}")
                    if b == 0:
                        nc.vector.tensor_add(zsum[:], zs[:, 0:1], zs[:, 1:2])
                    else:
                        zt0 = wp.tile([1, 1], f32, tag="zt0")
                        nc.vector.tensor_add(zt0[:], zs[:, 2:3], zs[:, 3:4])
                        nc.vector.tensor_add(zsum[:], zt0[:], zs[:, 4:5])
                    nc.vector.tensor_copy(zb[:, b: b + 1], zsum[:])
                    zS = wp.tile([1, 1], f32, tag=f"zS{b}")
                    nc.vector.tensor_scalar(out=zS[:], in0=zsum[:], scalar1=1.0 / S,
                                            scalar2=None, op0=ALU.mult)
                    nc.gpsimd.partition_broadcast(bcol[:, b: b + 1], zS[:])

                # broadcast e to all partitions via PE ones-matmul -> PSUM f32
                eB = ps.tile([128, 2048], f32, tag="eB")
                for j in range(ss // 512):
                    nc.tensor.matmul(
                        eB[:, 512 * j: 512 * (j + 1)],
                        ones16[:],
                        e16[:, 512 * j: 512 * (j + 1)],
                        start=True, stop=True,
                    )
                for k in range(NCHUNK):
                    if kind == 'z':
                        sc = bcol[:, b: b + 1]
                    else:
                        sc = 0.0
                    nc.vector.scalar_tensor_tensor(
                        out=scr[:, :ss],
                        in0=eB[:, :ss],
                        scalar=sc,
                        in1=xt[k][:, :ss].bitcast(f32),
                        op0=ALU.add,
                        op1=ALU.mult,
                        accum_out=ctx_e[:, 8 * ph + k: 8 * ph + k + 1],
                    )

            # ---- combine phases + normalization ----
            zbinv = wp.tile([1, B_PER_CORE], f32, tag="zbinv")
            nc.vector.reciprocal(zbinv[:], zb[:])
            zinv128 = wp.tile([128, B_PER_CORE], f32, tag="zinv")
            nc.gpsimd.partition_broadcast(zinv128[:], zbinv[:])

            # per-batch ctx_e sums over phases (b0: ph0+ph1; b1: ph2+ph3+ph4)
            ctxE = wp.tile([128, 2 * NCHUNK], f32, tag="ctxE")  # [b0 8 | b1 8]
            nc.vector.tensor_add(ctxE[:, :8], ctx_e[:, 0:8], ctx_e[:, 8:16])
            tE = wp.tile([128, NCHUNK], f32, tag="tE")
            nc.vector.tensor_add(tE[:], ctx_e[:, 16:24], ctx_e[:, 24:32])
            nc.vector.tensor_add(ctxE[:, 8:16], tE[:], ctx_e[:, 32:40])
            # uniform rowsums: b0 = ph0 only; b1 = ph2 + ph3
            ctxU1 = wp.tile([128, NCHUNK], f32, tag="ctxU1")
            nc.vector.tensor_add(ctxU1[:], ctx_u[:, 16:24], ctx_u[:, 24:32])

            # ctxA[:, 2k+b] = ctxE_b[:,k]/Z_b + ctxU_b[:,k]/S   (bf16 for w1)
            ctxA = wp.tile([128, 2 * NCHUNK], bf16, tag="ctxA")
            ctxEs = wp.tile([128, 2 * NCHUNK], f32, tag="ctxEs")
            for b in range(B_PER_CORE):
                nc.vector.tensor_scalar(
                    out=ctxEs[:, b::2], in0=ctxE[:, 8 * b: 8 * (b + 1)],
                    scalar1=zinv128[:, b: b + 1], scalar2=None, op0=ALU.mult,
                )
                nc.vector.scalar_tensor_tensor(
                    out=ctxA[:, b::2],
                    in0=(ctx_u[:, 0:8] if b == 0 else ctxU1[:]),
                    scalar=1.0 / S, in1=ctxEs[:, b::2],
                    op0=ALU.mult, op1=ALU.add,
                )

            # ---- MLP tail in [2, 64] layout ----
            t2p = ps.tile([B_PER_CORE, R], f32, tag="cm")
            for k in range(NCHUNK):
                nc.tensor.matmul(
                    t2p[:],
                    ctxA[:, 2 * k: 2 * k + 2],
                    w1t[:, R * k: R * (k + 1)],
                    start=(k == 0),
                    stop=(k == NCHUNK - 1),
                )
            t2 = wp.tile([B_PER_CORE, R], f32, tag="t2")
            nc.vector.tensor_add(t2[:], t2p[:], b1r)
            # LN over free dim: mean via ACT accum (scale folds the 1/R)
            junkr = wp.tile([B_PER_CORE, R], f32, tag="junkr")
            mu = wp.tile([B_PER_CORE, 1], f32, tag="mu")
            nc.scalar.activation(junkr[:], t2[:], AF.Copy, scale=1.0 / R,
                                 accum_out=mu[:])
            tctr = wp.tile([B_PER_CORE, R], f32, tag="tctr")
            nc.vector.tensor_scalar(out=tctr[:], in0=t2[:], scalar1=mu[:],
                                    scalar2=None, op0=ALU.subtract)
            vs = wp.tile([B_PER_CORE, 1], f32, tag="vs")
            nc.scalar.activation(junkr[:], tctr[:], AF.Square, scale=0.125,
                                 accum_out=vs[:])
            vep = wp.tile([B_PER_CORE, 1], f32, tag="vep")
            nc.vector.tensor_scalar(out=vep[:], in0=vs[:], scalar1=LN_EPS,
                                    scalar2=None, op0=ALU.add)
            std = wp.tile([B_PER_CORE, 1], f32, tag="std")
            nc.scalar.sqrt(std[:], vep[:])
            rstd = wp.tile([B_PER_CORE, 1], f32, tag="rstd")
            nc.vector.reciprocal(rstd[:], std[:])
            tn = wp.tile([B_PER_CORE, R], f32, tag="tn")
            nc.vector.tensor_scalar(out=tn[:], in0=tctr[:], scalar1=rstd[:],
                                    scalar2=None, op0=ALU.mult)
            tg = wp.tile([B_PER_CORE, R], f32, tag="tg")
            nc.vector.tensor_mul(tg[:], tn[:], lngr)
            tgb = wp.tile([B_PER_CORE, R], f32, tag="tgb")
            nc.vector.tensor_add(tgb[:], tg[:], lnbr)
            tr2 = wp.tile([B_PER_CORE, R], bf16, tag="tr2")
            nc.vector.tensor_scalar_max(tr2[:], tgb[:], 0.0)

            # transpose [2, 64] -> [64, 2] on PE, then w2 with bias row
            trTp = ps.tile([R, B_PER_CORE], bf16, tag="cm")
            nc.tensor.transpose(trTp[:], tr2[:], eye2[:])
            tr65 = wp.tile([R + 1, B_PER_CORE], bf16, tag="tr65")
            nc.vector.memset(tr65[R: R + 1, :], 1.0)
            nc.vector.tensor_copy(tr65[:R, :], trTp[:])

            outp = ps.tile([B_PER_CORE, C], f32, tag="cm")
            for h in range(C // 512):
                nc.tensor.matmul(
                    outp[:, 512 * h: 512 * (h + 1)],
                    tr65[:],
                    w2tb[:, 512 * h: 512 * (h + 1)],
                    start=True,
                    stop=True,
                )
            out_sb = wp.tile([B_PER_CORE, C], f32, tag="out_sb")
            nc.vector.tensor_copy(out_sb[:, 0:512], outp[:, 0:512])
            nc.scalar.copy(out_sb[:, 512:1024], outp[:, 512:1024])
            nc.sync.dma_start(out_d[:], out_sb[:])

    nc.compile()
    return nc


def _prep_inputs(x, wm, w1, b1, ln_g, ln_b, w2, b2):
    from ml_dtypes import bfloat16
    x = np.ascontiguousarray(x, dtype=np.float32).reshape(16, C, S)
    wmT = np.ascontiguousarray(wm.astype(np.float32).reshape(NCHUNK, 128).T)
    # w1t[p, 64k+r] = w1[r, 128k+p]
    w1t = np.ascontiguousarray(
        w1.astype(np.float32).reshape(R, NCHUNK, 128).transpose(2, 1, 0).reshape(128, NCHUNK * R)
    ).astype(bfloat16)
    w2tb = np.concatenate(
        [w2.astype(np.float32).T, b2.astype(np.float32)[None, :]], axis=0
    )
    w2tb = np.ascontiguousarray(w2tb).astype(bfloat16)
    vecs = np.stack([
        np.concatenate([b1, ln_g, ln_b]).astype(np.float32)
    ] * B_PER_CORE, axis=0)
    vecs = np.ascontiguousarray(vecs)
    eye2 = np.eye(B_PER_CORE).astype(bfloat16)
    in_maps = []
    for c in range(N_CORES):
        in_maps.append(
            {
                "x": x[B_PER_CORE * c: B_PER_CORE * (c + 1)],
                "wmT": wmT,
                "w1t": w1t,
                "w2tb": w2tb,
                "vecs": vecs,
                "eye2": eye2,
            }
        )
    return in_maps


def _run(inputs, trace=False, trace_kwargs=None, tmpdir=None):
    from concourse.bass_utils import run_bass_kernel_spmd

    if "nc" not in _CACHE:
        _CACHE["nc"] = _build()
    nc = _CACHE["nc"]
    in_maps = _prep_inputs(
        inputs["x"], inputs["wm"], inputs["w1"], inputs["b1"],
        inputs["ln_g"], inputs["ln_b"], inputs["w2"], inputs["b2"],
    )
    br = run_bass_kernel_spmd(
        nc, in_maps, list(range(N_CORES)), trace=trace,
        trace_kwargs=trace_kwargs or {}, tmpdir=tmpdir,
    )
    out = np.concatenate([np.asarray(r["out"]) for r in br.results], axis=0)
    return out.reshape(16, C, 1, 1).astype(np.float32), br


def kernel(x, wm, bm, w1, b1, ln_g, ln_b, w2, b2):
    inputs = dict(x=x, wm=wm, bm=bm, w1=w1, b1=b1, ln_g=ln_g, ln_b=ln_b, w2=w2, b2=b2)
    out, _ = _run({k: np.asarray(v) for k, v in inputs.items()})
    return out


# revision 12
# speedup vs baseline: 1.0745x; 1.0745x over previous
"""Trainium2 Bass kernel for nn_DGC_Attention (global-context attention block).

Math (per batch b):
    cm[s]   = sum_c x[b,c,s] * wm[c]            (+ bm, which cancels in softmax)
    mask[s] = softmax(cm)[s] + 1/S              (uniform part: softmax of zeros)
    ctx[c]  = sum_s x[b,c,s] * mask[s]
    t       = relu(LN(ctx @ w1.T + b1) * ln_g + ln_b)
    out     = t @ w2.T + b2                     -> [B, C, 1, 1]

Sharding: pure data parallel, batch dim (16) over 8 cores, 2 batches/core.

Pipeline (v3): x streams in (b, s-slice) phases: b0 [2048u, 2048z],
b1 [2048u, 1024u, 1024z].  Per phase:
    - 8 chunk DMAs [128, ss] fp32 (HWDGE single queue; 1MB / 0.5MB each).
    - PE: cm partial via f32r matmuls -> [1, ss] PSUM.
    - 'u' phases: ACT per chunk Copy -> junk bf16 with accum_out = rowsum
      column (uniform part), as chunks arrive; fully hidden under DMA.
    - ACT phase end: Exp over PSUM row -> e bf16 [1, ss], accum_out = Z.
    - PE: broadcast e to eB [128, ss] f32 in PSUM via ones-matmul (fast,
      replaces the slow gpsimd partition_broadcast).
    - DVE per chunk: stt (eB + beta) * x, accum_out -> ctx_e column.
      'z' phases (the last slice of each batch) use beta = Z_batch/S as a
      per-partition scalar AP, folding the uniform part into the same pass
      (no rowsums); 'u' phases use beta = 0.
    - The final z-phase splits its 8 chunk-stts 6 on DVE + 2 on GPSIMD to
      shorten the post-DMA backlog.
Tail: ctx = ctx_e/Z (+ rowsums/S for u phases), tiny MLP in [2, 64] layout
(LN stats via ACT accum), PE transpose, w2 matmul with bias as 65th row.
"""
import numpy as np

B_PER_CORE = 2
N_CORES = 8
C = 1024
S = 4096
R = 64
NCHUNK = C // 128           # 8 c-chunks
LN_EPS = 1e-5

# (batch, s0, ss, kind, eb0) kind: 'u' = rowsum, 'z' = Z/S-folded;
# eb0 = column offset into the shared eB PSUM tile (late phases use disjoint
# ranges so the next phase's broadcast has no WAR on the previous stts)
PHASES = [
    (0, 0, 2048, 'u', 0), (0, 2048, 2048, 'z', 0),
    (1, 0, 2048, 'u', 0), (1, 2048, 1536, 'u', 0), (1, 3584, 512, 'z', 1536),
]
NPH = len(PHASES)
N_GPS = 0                   # gpsimd cannot run stt on trn2 (Pool ISA rejects it)

_CACHE = {}


def _build():
    import concourse.bass as bass
    import concourse.tile as tile
    from concourse import bacc, mybir, bass_isa

    f32 = mybir.dt.float32
    f32r = mybir.dt.float32r
    bf16 = mybir.dt.bfloat16
    AF = mybir.ActivationFunctionType
    ALU = mybir.AluOpType

    nc = bacc.Bacc("TRN2", target_bir_lowering=False, debug=False, num_devices=N_CORES)

    x_d = nc.dram_tensor("x", [B_PER_CORE, C, S], f32, kind="ExternalInput").ap()
    wmT_d = nc.dram_tensor("wmT", [128, NCHUNK], f32, kind="ExternalInput").ap()
    w1t_d = nc.dram_tensor("w1t", [128, NCHUNK * R], bf16, kind="ExternalInput").ap()
    # w2tb[r, c] for r<64 = w2[c, r]; row 64 = b2  (bias folded into matmul)
    w2tb_d = nc.dram_tensor("w2tb", [R + 1, C], bf16, kind="ExternalInput").ap()
    # [2, 3R] broadcast rows: b1 | ln_g | ln_b
    vecs_d = nc.dram_tensor("vecs", [B_PER_CORE, 3 * R], f32, kind="ExternalInput").ap()
    eye_d = nc.dram_tensor("eye2", [B_PER_CORE, B_PER_CORE], bf16, kind="ExternalInput").ap()
    out_d = nc.dram_tensor("out", [B_PER_CORE, C], f32, kind="ExternalOutput").ap()

    with tile.TileContext(nc) as tc:
        with (
            tc.tile_pool(name="xp", bufs=20) as xp,
            tc.tile_pool(name="cp", bufs=1) as cp,
            tc.tile_pool(name="wp", bufs=1) as wp,
            tc.tile_pool(name="ps", bufs=1, space="PSUM") as ps,
        ):
            # weights: wmT first (tiny, unblocks PE), rest after
            wmT = cp.tile([128, NCHUNK], f32r, tag="wmT")
            nc.gpsimd.dma_start(wmT[:], wmT_d.bitcast(f32r))
            w1t = cp.tile([128, NCHUNK * R], bf16, tag="w1t")
            nc.gpsimd.dma_start(w1t[:], w1t_d)
            w2tb = cp.tile([R + 1, C], bf16, tag="w2tb")
            nc.gpsimd.dma_start(w2tb[:], w2tb_d)
            vecs = cp.tile([B_PER_CORE, 3 * R], f32, tag="vecs")
            nc.gpsimd.dma_start(vecs[:], vecs_d)
            eye2 = cp.tile([B_PER_CORE, B_PER_CORE], bf16, tag="eye2")
            nc.gpsimd.dma_start(eye2[:], eye_d)
            b1r = vecs[:, 0:R]
            lngr = vecs[:, R:2 * R]
            lnbr = vecs[:, 2 * R:3 * R]

            # accumulator columns, col = 8*ph + k
            ctx_e = wp.tile([128, NPH * NCHUNK], f32, tag="ctx_e")
            ctx_u = wp.tile([128, NPH * NCHUNK], f32, tag="ctx_u")
            zs = wp.tile([1, NPH], f32, tag="zs")
            zb = wp.tile([1, B_PER_CORE], f32, tag="zb")
            bcol = wp.tile([128, B_PER_CORE], f32, tag="bcol")

            ones16 = wp.tile([1, 128], bf16, tag="ones16")
            nc.vector.memset(ones16[:], 1.0)

            # PE warm-up (fp32 so any-N is legal) + ACT table warm-ups
            dum = ps.tile([1, 1], f32, tag="cm")
            nc.tensor.matmul(dum[:], wmT[:, :1].bitcast(f32), wmT[:, :1].bitcast(f32),
                             start=True, stop=True)
            ewarm = wp.tile([1, 1], f32, tag="ewarm")
            nc.scalar.activation(ewarm[:], zs[:, :1], AF.Exp)
            swarm = wp.tile([1, 1], f32, tag="swarm")
            nc.scalar.sqrt(swarm[:], ewarm[:])

            scr = wp.tile([128, 2048], bf16, tag="scr")    # DVE stt junk out
            junk = wp.tile([128, 2048], bf16, tag="junk")  # ACT rowsum junk out

            for ph, (b, s0, ss, kind, eb0) in enumerate(PHASES):
                last = (ph == NPH - 1)
                xt = []
                for k in range(NCHUNK):
                    t = xp.tile([128, 2048], f32r, tag="x")
                    nc.sync.dma_start(
                        t[:, :ss],
                        x_d[b, 128 * k: 128 * (k + 1), s0: s0 + ss].bitcast(f32r),
                    )
                    xt.append(t)

                cm = ps.tile([1, 2048], f32, tag="cm")
                for k in range(NCHUNK):
                    for j in range(ss // 512):
                        nc.tensor.matmul(
                            cm[:, 512 * j: 512 * (j + 1)],
                            wmT[:, k: k + 1],
                            xt[k][:, 512 * j: 512 * (j + 1)],
                            start=(k == 0),
                            stop=(k == NCHUNK - 1),
                        )
                    if kind == 'u':
                        # uniform-part rowsum, as chunks arrive
                        nc.scalar.activation(
                            junk[:, :ss], xt[k][:, :ss].bitcast(f32), AF.Copy,
                            accum_out=ctx_u[:, 8 * ph + k: 8 * ph + k + 1],
                        )

                e16 = wp.tile([1, 2048], bf16, tag=f"e16_{ph % 2}")
                nc.scalar.activation(
                    e16[:, :ss], cm[:, :ss], AF.Exp,
                    accum_out=zs[:, ph: ph + 1],
                )
                if kind == 'z':
                    # beta = Z_batch / S, as a [128,1] scalar column
                    zsum = wp.tile([1, 1], f32, tag=f"zsum{b}")
                    if b == 0:
                        nc.vector.tensor_add(zsum[:], zs[:, 0:1], zs[:, 1:2])
                    else:
                        zt0 = wp.tile([1, 1], f32, tag="zt0")
                        nc.vector.tensor_add(zt0[:], zs[:, 2:3], zs[:, 3:4])
                        nc.vector.tensor_add(zsum[:], zt0[:], zs[:, 4:5])
                    nc.vector.tensor_copy(zb[:, b: b + 1], zsum[:])
                    zS = wp.tile([1, 1], f32, tag=f"zS{b}")
                    nc.vector.tensor_scalar(out=zS[:], in0=zsum[:], scalar1=1.0 / S,
                                            scalar2=None, op0=ALU.mult)
                    nc.gpsimd.partition_broadcast(bcol[:, b: b + 1], zS[:])

                # broadcast e to all partitions via PE ones-matmul -> PSUM f32
                if eb0 == 0:
                    eB = ps.tile([128, 2048], f32, tag="eB")
                eBs = eB[:, eb0: eb0 + ss]
                for j in range(ss // 512):
                    nc.tensor.matmul(
                        eBs[:, 512 * j: 512 * (j + 1)],
                        ones16[:],
                        e16[:, 512 * j: 512 * (j + 1)],
                        start=True, stop=True,
                    )
                if last:
                    # hide the Sqrt ACT-table load under the stt backlog
                    nc.scalar.sqrt(swarm[:], ewarm[:])
                for k in range(NCHUNK):
                    if kind == 'z':
                        sc = bcol[:, b: b + 1]
                    else:
                        sc = 0.0
                    nc.vector.scalar_tensor_tensor(
                        out=scr[:, :ss],
                        in0=eBs[:],
                        scalar=sc,
                        in1=xt[k][:, :ss].bitcast(f32),
                        op0=ALU.add,
                        op1=ALU.mult,
                        accum_out=ctx_e[:, 8 * ph + k: 8 * ph + k + 1],
                    )

            # ---- combine phases + normalization ----
            zbinv = wp.tile([1, B_PER_CORE], f32, tag="zbinv")
            nc.vector.reciprocal(zbinv[:], zb[:])
            zinv128 = wp.tile([128, B_PER_CORE], f32, tag="zinv")
            nc.gpsimd.partition_broadcast(zinv128[:], zbinv[:])

            # per-batch ctx_e sums over phases (b0: ph0+ph1; b1: ph2+ph3+ph4)
            ctxE = wp.tile([128, 2 * NCHUNK], f32, tag="ctxE")  # [b0 8 | b1 8]
            nc.vector.tensor_add(ctxE[:, :8], ctx_e[:, 0:8], ctx_e[:, 8:16])
            tE = wp.tile([128, NCHUNK], f32, tag="tE")
            nc.vector.tensor_add(tE[:], ctx_e[:, 16:24], ctx_e[:, 24:32])
            nc.vector.tensor_add(ctxE[:, 8:16], tE[:], ctx_e[:, 32:40])
            # uniform rowsums: b0 = ph0 only; b1 = ph2 + ph3
            ctxU1 = wp.tile([128, NCHUNK], f32, tag="ctxU1")
            nc.vector.tensor_add(ctxU1[:], ctx_u[:, 16:24], ctx_u[:, 24:32])

            # ctxA[:, 2k+b] = ctxE_b[:,k]/Z_b + ctxU_b[:,k]/S   (bf16 for w1)
            ctxA = wp.tile([128, 2 * NCHUNK], bf16, tag="ctxA")
            ctxEs = wp.tile([128, 2 * NCHUNK], f32, tag="ctxEs")
            for b in range(B_PER_CORE):
                nc.vector.tensor_scalar(
                    out=ctxEs[:, b::2], in0=ctxE[:, 8 * b: 8 * (b + 1)],
                    scalar1=zinv128[:, b: b + 1], scalar2=None, op0=ALU.mult,
                )
                nc.vector.scalar_tensor_tensor(
                    out=ctxA[:, b::2],
                    in0=(ctx_u[:, 0:8] if b == 0 else ctxU1[:]),
                    scalar=1.0 / S, in1=ctxEs[:, b::2],
                    op0=ALU.mult, op1=ALU.add,
                )

            # ---- MLP tail in [2, 64] layout ----
            t2p = ps.tile([B_PER_CORE, R], f32, tag="cm")
            for k in range(NCHUNK):
                nc.tensor.matmul(
                    t2p[:],
                    ctxA[:, 2 * k: 2 * k + 2],
                    w1t[:, R * k: R * (k + 1)],
                    start=(k == 0),
                    stop=(k == NCHUNK - 1),
                )
            t2 = wp.tile([B_PER_CORE, R], f32, tag="t2")
            nc.vector.tensor_add(t2[:], t2p[:], b1r)
            # LN over free dim, stats on DVE (accum ops); ACT only does sqrt
            junkr = wp.tile([B_PER_CORE, R], f32, tag="junkr")
            mu = wp.tile([B_PER_CORE, 1], f32, tag="mu")
            nc.vector.tensor_scalar(out=junkr[:], in0=t2[:], scalar1=1.0 / R,
                                    scalar2=0.0, op0=ALU.mult, op1=ALU.add,
                                    accum_out=mu[:])
            tctr = wp.tile([B_PER_CORE, R], f32, tag="tctr")
            nc.vector.tensor_scalar(out=tctr[:], in0=t2[:], scalar1=mu[:],
                                    scalar2=None, op0=ALU.subtract)
            vs = wp.tile([B_PER_CORE, 1], f32, tag="vs")
            nc.vector.scalar_tensor_tensor(out=junkr[:], in0=tctr[:], scalar=1.0,
                                           in1=tctr[:], op0=ALU.mult, op1=ALU.mult,
                                           accum_out=vs[:])
            vep = wp.tile([B_PER_CORE, 1], f32, tag="vep")
            nc.vector.tensor_scalar(out=vep[:], in0=vs[:], scalar1=1.0 / R,
                                    scalar2=LN_EPS, op0=ALU.mult, op1=ALU.add)
            std = wp.tile([B_PER_CORE, 1], f32, tag="std")
            nc.scalar.sqrt(std[:], vep[:])
            rstd = wp.tile([B_PER_CORE, 1], f32, tag="rstd")
            nc.vector.reciprocal(rstd[:], std[:])
            tn = wp.tile([B_PER_CORE, R], f32, tag="tn")
            nc.vector.tensor_scalar(out=tn[:], in0=tctr[:], scalar1=rstd[:],
                                    scalar2=None, op0=ALU.mult)
            tg = wp.tile([B_PER_CORE, R], f32, tag="tg")
            nc.vector.tensor_mul(tg[:], tn[:], lngr)
            tgb = wp.tile([B_PER_CORE, R], f32, tag="tgb")
            nc.vector.tensor_add(tgb[:], tg[:], lnbr)
            tr2 = wp.tile([B_PER_CORE, R], bf16, tag="tr2")
            nc.vector.tensor_scalar_max(tr2[:], tgb[:], 0.0)

            # transpose [2, 64] -> [64, 2] on PE, then w2 with bias row
            trTp = ps.tile([R, B_PER_CORE], bf16, tag="cm")
            nc.tensor.transpose(trTp[:], tr2[:], eye2[:])
            tr65 = wp.tile([R + 1, B_PER_CORE], bf16, tag="tr65")
            nc.vector.memset(tr65[R: R + 1, :], 1.0)
            nc.vector.tensor_copy(tr65[:R, :], trTp[:])

            outp = ps.tile([B_PER_CORE, C], f32, tag="cm")
            for h in range(C // 512):
                nc.tensor.matmul(
                    outp[:, 512 * h: 512 * (h + 1)],
                    tr65[:],
                    w2tb[:, 512 * h: 512 * (h + 1)],
                    start=True,
                    stop=True,
                )
            out_sb = wp.tile([B_PER_CORE, C], f32, tag="out_sb")
            nc.vector.tensor_copy(out_sb[:, 0:512], outp[:, 0:512])
            nc.scalar.copy(out_sb[:, 512:1024], outp[:, 512:1024])
            nc.sync.dma_start(out_d[:], out_sb[:])

    nc.compile()
    return nc


def _prep_inputs(x, wm, w1, b1, ln_g, ln_b, w2, b2):
    from ml_dtypes import bfloat16
    x = np.ascontiguousarray(x, dtype=np.float32).reshape(16, C, S)
    wmT = np.ascontiguousarray(wm.astype(np.float32).reshape(NCHUNK, 128).T)
    # w1t[p, 64k+r] = w1[r, 128k+p]
    w1t = np.ascontiguousarray(
        w1.astype(np.float32).reshape(R, NCHUNK, 128).transpose(2, 1, 0).reshape(128, NCHUNK * R)
    ).astype(bfloat16)
    w2tb = np.concatenate(
        [w2.astype(np.float32).T, b2.astype(np.float32)[None, :]], axis=0
    )
    w2tb = np.ascontiguousarray(w2tb).astype(bfloat16)
    vecs = np.stack([
        np.concatenate([b1, ln_g, ln_b]).astype(np.float32)
    ] * B_PER_CORE, axis=0)
    vecs = np.ascontiguousarray(vecs)
    eye2 = np.eye(B_PER_CORE).astype(bfloat16)
    in_maps = []
    for c in range(N_CORES):
        in_maps.append(
            {
                "x": x[B_PER_CORE * c: B_PER_CORE * (c + 1)],
                "wmT": wmT,
                "w1t": w1t,
                "w2tb": w2tb,
                "vecs": vecs,
                "eye2": eye2,
            }
        )
    return in_maps


def _run(inputs, trace=False, trace_kwargs=None, tmpdir=None):
    from concourse.bass_utils import run_bass_kernel_spmd

    if "nc" not in _CACHE:
        _CACHE["nc"] = _build()
    nc = _CACHE["nc"]
    in_maps = _prep_inputs(
        inputs["x"], inputs["wm"], inputs["w1"], inputs["b1"],
        inputs["ln_g"], inputs["ln_b"], inputs["w2"], inputs["b2"],
    )
    br = run_bass_kernel_spmd(
        nc, in_maps, list(range(N_CORES)), trace=trace,
        trace_kwargs=trace_kwargs or {}, tmpdir=tmpdir,
    )
    out = np.concatenate([np.asarray(r["out"]) for r in br.results], axis=0)
    return out.reshape(16, C, 1, 1).astype(np.float32), br


def kernel(x, wm, bm, w1, b1, ln_g, ln_b, w2, b2):
    inputs = dict(x=x, wm=wm, bm=bm, w1=w1, b1=b1, ln_g=ln_g, ln_b=ln_b, w2=w2, b2=b2)
    out, _ = _run({k: np.asarray(v) for k, v in inputs.items()})
    return out
